# revision 40
# baseline (speedup 1.0000x reference)
"""Trainium2 Bass kernel for batched single-head attention with seq-sum pooling.

Reference computation (B=16, S=2048, D=512, fp32):
    q = x @ W_q ; k = x @ W_k ; v = x @ W_v          per batch  [S, D]
    scores = q @ k.T / sqrt(D)                        [S, S]
    attn = softmax(scores, axis=-1)
    out_b = sum_s (attn @ v)[s, :]                    [D]

Algebraic restructures (carried over from the bf16 version):
1. The final sum over query positions commutes through both trailing
   matmuls: out_b = ((r^T E) @ x) @ W_v with E = exp(scores/sqrt(D)) and
   r[q] = 1/rowsum_q(E) — removes the [S,S]x[S,D] attention-value matmul
   AND the V projection.
2. scores = x M x^T with M = W_q W_k^T computed once per core — replaces
   both per-batch Q/K projections with a single G = x M projection.

fp8 DoubleRow on the three dominant matmul groups (scores, G projection,
colsum) virtualizes the PE to K=256 (two fp8 weights per cell):
- x^T and 16*M are stored fp8; G's PSUM result (16*G) downcasts to fp8.
- scores PSUM holds 16*(q.k); the exp activation folds the 1/16 into its
  scale and applies bias -2 so E = exp(s/sqrt(D) - 2) stays within TRN
  fp8e4's +-240 range (softmax normalization cancels the shift exactly).
- E lands fp8 in PAIRED q-tile tiles [P, 2, S] so the colsum runs as
  DoubleRow too; the softmax reciprocal r is scaled by 256 (fp8 subnormal
  floor) and divided back out in the epilogue copy.
- The y = w @ X matvec and epilogue stay bf16 (x natural in fp8 would put
  ~3.6% noise directly on the output path).  rel err ~8e-3 (tol 2e-2).

PE/ACT pipeline shape (the v1 fp8 kernel measured 103 DR matmuls paying a
serialized 213ns LDWEIGHTS and an ACT busy near the PE's):
- Scores run ep-OUTER per q-tile: one 256-col LDWEIGHTS per gt stationary,
  then 4 kc matmuls reusing it, into two [P, 1024] PSUM tiles (kc pairs).
- exp runs once per kc-pair ([P, 1024] across two PSUM banks), halving the
  per-instruction ACT fixed cost + accumulator-read count.
- PSUM is exactly 8 banks: scores 2x[P,1024] + gp 2x[P,512] + a single
  [P,1024] w slot.  The colsum accumulator therefore only holds kc 0-1
  during the q-tile loop; kc 2-3 run as a second sweep over the persistent
  fp8 E tiles, woven into the next phase alongside the w-phase (whose y
  accumulator rides the same w slot after the sweep drains).
- G projection reuses each 16M stationary across an s-chunk pair.

Sharding: pure data parallelism over batch — 2 batch elements per core on 8
NeuronCores, weights replicated, no collectives.  Host concatenates per-core
[2, D] outputs.
"""

import sys

sys.path.insert(0, "/opt/trn_rl_repo")

import numpy as np

import concourse.bass as bass
import concourse.mybir as mybir
import concourse.tile as tile
from concourse import bacc
from concourse.bass_utils import run_bass_kernel_spmd
from concourse.masks import make_identity

B, S, D = 16, 2048, 512
P = 128
N_CORES = 8
B_PER_CORE = B // N_CORES  # 2
SCALE = 1.0 / float(np.sqrt(D))

F32 = mybir.dt.float32
BF16 = mybir.dt.bfloat16
F8 = mybir.dt.float8e4
DR = mybir.MatmulPerfMode.DoubleRow
EXP = mybir.ActivationFunctionType.Exp

M_SCALE = 16.0  # M stored as 16*M in fp8 (keeps values in normal range)
R_SCALE = 256.0  # r stored as 256*r in fp8 (dodges the subnormal floor)
EXP_BIAS = -2.0  # E = exp(s/sqrt(D) - 2): max stays well under fp8e4's 240

N_ST = S // P  # 16 s-tiles (partition tiles of the sequence dim)
N_DT = D // P  # 4 d-tiles (partition tiles of the feature dim)
N_DP = N_DT // 2  # 2 DoubleRow d-tile pairs
NCH = 512  # one fp32 PSUM bank of moving free dim
N_SC = S // NCH  # 4 s-chunks of the sequence dim
N_KC = S // NCH  # 4 k-chunks of the key dim
N_QP = N_ST // 2  # 8 q-tile pairs (DoubleRow colsum granularity)


def build_nc():
    nc = bacc.Bacc("TRN2", target_bir_lowering=False, debug=False, num_devices=N_CORES)
    x_ext = nc.dram_tensor(
        "inputs", [B_PER_CORE, S, D], F32, kind="ExternalInput"
    ).ap()
    wq_ext = nc.dram_tensor("W_q", [D, D], F32, kind="ExternalInput").ap()
    wk_ext = nc.dram_tensor("W_k", [D, D], F32, kind="ExternalInput").ap()
    wv_ext = nc.dram_tensor("W_v", [D, D], F32, kind="ExternalInput").ap()
    out_ext = nc.dram_tensor("out", [B_PER_CORE, D], F32, kind="ExternalOutput").ap()

    with tile.TileContext(nc) as tc:
        with (
            tc.tile_pool(name="const", bufs=1) as const_pool,
            tc.tile_pool(name="w", bufs=1) as w_pool,
            tc.tile_pool(name="xnat", bufs=2) as xnat_pool,
            tc.tile_pool(name="xt", bufs=2) as xt_pool,
            tc.tile_pool(name="qkv", bufs=2) as qkv_pool,
            tc.tile_pool(name="e", bufs=10) as e_pool,
            tc.tile_pool(name="soft", bufs=4) as soft_pool,
            tc.tile_pool(name="wvec", bufs=2) as wvec_pool,
            tc.tile_pool(name="scps", bufs=2, space="PSUM") as sc_psum,
            tc.tile_pool(name="gpps", bufs=2, space="PSUM") as gp_psum,
            tc.tile_pool(name="wps", bufs=1, space="PSUM") as w_psum,
        ):
            one_t = const_pool.tile([1, 1], BF16)
            nc.gpsimd.memset(one_t[:], 1.0)
            ident_f = const_pool.tile([P, P], F32)
            make_identity(nc, ident_f[:])
            ident = const_pool.tile([P, P], BF16)
            nc.vector.tensor_copy(ident[:], ident_f[:])
            ebias_t = const_pool.tile([P, 1], F32)
            nc.vector.memset(ebias_t[:], EXP_BIAS)

            # x arrives via SWDGE cast-DMA (f32 -> bf16) into natural-layout
            # staging tiles; the transpose to xT happens on the PE as a
            # REGULAR identity matmul (out = x_tile.T @ I) whose PSUM result
            # downcasts straight to fp8.
            def dma_x_chunk(b, sc, xnat_s):
                nc.gpsimd.dma_start(
                    out=xnat_s[:, sc * 4 : (sc + 1) * 4, :],
                    in_=x_ext[b, sc * NCH : (sc + 1) * NCH, :].rearrange(
                        "(t p) d -> p t d", p=P
                    ),
                )

            w_tiles = {}

            def dma_w(name, ext):
                w_s = w_pool.tile([P, N_DT, D], BF16, tag=name)
                nc.gpsimd.dma_start(
                    out=w_s[:], in_=ext.rearrange("(t p) e -> p t e", p=P)
                )
                w_tiles[name] = w_s

            # Batch 0's x chunks and the weight loads share the SWDGE queue;
            # order so each lands just before the PE needs it.
            xnat0_s = xnat_pool.tile([P, N_ST, D], BF16, tag="xnat")
            x0_loaded = [False] * N_SC
            # s-tile 0 rides the parallel HWDGE queue as f32 (lands ~3us
            # before the SWDGE cast chain's first byte); its transposes run
            # as fp32 identity matmuls.
            xf0 = xnat_pool.tile([P, D], F32, tag="xf0")
            nc.sync.dma_start(out=xf0[:], in_=x_ext[0, 0:P, :])
            nc.vector.tensor_copy(xnat0_s[:, 0, :], xf0[:])
            nc.gpsimd.dma_start(
                out=xnat0_s[:, 1:4, :],
                in_=x_ext[0, P:NCH, :].rearrange("(t p) d -> p t d", p=P),
            )
            x0_loaded[0] = True
            dma_w("wk", wk_ext)
            dma_w("wq", wq_ext)
            dma_x_chunk(0, 1, xnat0_s)
            x0_loaded[1] = True
            dma_x_chunk(0, 2, xnat0_s)
            x0_loaded[2] = True
            dma_x_chunk(0, 3, xnat0_s)
            x0_loaded[3] = True
            dma_w("wv", wv_ext)
            wk_s, wq_s, wv_s = w_tiles["wk"], w_tiles["wq"], w_tiles["wv"]

            # One-time prework: scores = (X Wq)(X Wk)^T = X M X^T with
            # M = Wq Wk^T [D, D].  M is computed in bf16 and stored as
            # 16*M in fp8 for the DoubleRow G projection.
            wqT_s = w_pool.tile([P, N_DT, D], BF16, tag="wqT")
            wkT_s = w_pool.tile([P, N_DT, D], BF16, tag="wkT")
            m_s = w_pool.tile([P, N_DT, D], F8, tag="m")

            def m_prework_thunks():
                thunks = []

                def make_wtrans_unit(src_w, dst, t_e):
                    def th():
                        tp = sc_psum.tile([P, N_DT * P], F32, tag="sc")
                        for t_a in range(N_DT):
                            nc.tensor.matmul(
                                tp[:, t_a * P : (t_a + 1) * P],
                                src_w[:, t_a, t_e * P : (t_e + 1) * P],
                                ident[:],
                                start=True,
                                stop=True,
                                skip_group_check=True,
                            )
                        nc.vector.tensor_copy(
                            dst[:, t_e, :],
                            tp[:],
                        )

                    return th

                def make_m_group(t_a):
                    def th():
                        mp = gp_psum.tile([P, NCH], F32, tag="gp")
                        for t_e in range(N_DT):
                            nc.tensor.matmul(
                                mp[:],
                                wqT_s[:, t_e, t_a * P : (t_a + 1) * P],
                                wkT_s[:, t_e, :],
                                start=(t_e == 0),
                                stop=(t_e == N_DT - 1),
                            )
                        nc.scalar.mul(m_s[:, t_a, :], mp[:], M_SCALE)

                    return th

                for t_e in range(N_DT):
                    thunks.append(make_wtrans_unit(wk_s, wkT_s, t_e))
                for t_e in range(N_DT):
                    thunks.append(make_wtrans_unit(wq_s, wqT_s, t_e))
                for t_a in range(N_DT):
                    thunks.append(make_m_group(t_a))
                return thunks

            # ---------- thunk builders (emission deferred for interleaving) --

            def proj_thunks(b, xnat_s, loaded):
                """Transpose + G = X M projection thunks for batch b.
                xt and gt land in fp8; the G PSUM holds 16*G (from 16*M).
                G groups cover an s-chunk PAIR so each 16M DoubleRow
                stationary is loaded once per two matmuls."""
                xt_s = xt_pool.tile([P, N_DT, S], F8, tag="xt")
                gt_s = qkv_pool.tile([P, N_DT, S], F8, tag="gt")

                def make_dma(sc):
                    def th():
                        dma_x_chunk(b, sc, xnat_s)

                    return th

                dma_th = [
                    None if loaded[sc] else make_dma(sc) for sc in range(N_SC)
                ]

                def make_trans_unit(sc, t_i, ring="sc", eng="act"):
                    def th():
                        st = sc * 4 + t_i
                        pool = sc_psum if ring == "sc" else gp_psum
                        tp = pool.tile([P, N_DT * P], F32, tag=ring, name="ttp")
                        for dt_i in range(N_DT):
                            nc.tensor.matmul(
                                tp[:, dt_i * P : (dt_i + 1) * P],
                                xnat_s[:, st, dt_i * P : (dt_i + 1) * P],
                                ident[:],
                                start=True,
                                stop=True,
                                skip_group_check=True,
                            )
                        # ACT drains in the projection phases (DVE handles
                        # the G copies there); DVE when woven into the
                        # ACT-bound scores phases.
                        e = nc.scalar.copy if eng == "act" else nc.vector.tensor_copy
                        e(
                            xt_s[:, :, st * P : (st + 1) * P],
                            tp[:].rearrange("p (t c) -> p t c", t=N_DT),
                        )

                    return th

                trans_th = [
                    [make_trans_unit(sc, t_i) for t_i in range(4)]
                    for sc in range(N_SC)
                ]

                def make_g(scp, ct):
                    def th():
                        mpa = gp_psum.tile([P, NCH], F32, tag="gp")
                        mpb = gp_psum.tile([P, NCH], F32, tag="gp")
                        sca, scb = 2 * scp, 2 * scp + 1
                        for kp in range(N_DP):
                            st_w = m_s[:, 2 * kp : 2 * kp + 2, ct * P : (ct + 1) * P]
                            nc.tensor.matmul(
                                mpa[:],
                                st_w,
                                xt_s[:, 2 * kp : 2 * kp + 2, sca * NCH : (sca + 1) * NCH],
                                start=(kp == 0),
                                stop=(kp == N_DP - 1),
                                perf_mode=DR,
                            )
                            nc.tensor.matmul(
                                mpb[:],
                                st_w,
                                xt_s[:, 2 * kp : 2 * kp + 2, scb * NCH : (scb + 1) * NCH],
                                start=(kp == 0),
                                stop=(kp == N_DP - 1),
                                perf_mode=DR,
                            )
                        nc.vector.tensor_copy(
                            gt_s[:, ct, sca * NCH : (sca + 1) * NCH], mpa[:]
                        )
                        nc.vector.tensor_copy(
                            gt_s[:, ct, scb * NCH : (scb + 1) * NCH], mpb[:]
                        )

                    return th

                kq_th = [
                    [make_g(scp, ct) for ct in range(N_DT)]
                    for scp in range(N_SC // 2)
                ]
                return (gt_s, xt_s), dma_th, trans_th, kq_th, make_trans_unit, make_g

            def emit_ltp(dma_th, trans_th, kq_th, extra=None):
                """Emit the transpose/projection stream.  G groups span
                s-chunk pairs, so the order is: transposes for chunks
                2p,2p+1 -> G groups for pair p woven with the next pair's
                transposes.  `extra` thunks are spread evenly over all
                emission slots."""
                extra = list(extra) if extra else []
                slots = []  # flat list of thunks in emission order

                def emit_pair_dmas(p):
                    for j in (2 * p, 2 * p + 1):
                        if j < N_SC and dma_th[j] is not None:
                            dma_th[j]()
                            dma_th[j] = None

                emit_pair_dmas(0)
                for th in trans_th[0] + trans_th[1]:
                    slots.append(th)
                for scp in range(N_SC // 2):
                    if scp + 1 < N_SC // 2:
                        nxt = trans_th[2 * scp + 2] + trans_th[2 * scp + 3]
                    else:
                        nxt = []
                    groups = list(kq_th[scp])
                    ti = 0
                    for g_i, g in enumerate(groups):
                        if g_i == 0:
                            slots.append(lambda p=scp + 1: emit_pair_dmas(p))
                        slots.append(g)
                        while ti < len(nxt) and ti * len(groups) < (g_i + 1) * len(nxt):
                            slots.append(nxt[ti])
                            ti += 1
                    while ti < len(nxt):
                        slots.append(nxt[ti])
                        ti += 1

                n_slots = len(slots)
                ei = 0
                for s_i, th in enumerate(slots):
                    th()
                    # spread extras evenly across remaining slots
                    want = ((s_i + 1) * len(extra)) // n_slots
                    while ei < want:
                        extra[ei]()
                        ei += 1
                while ei < len(extra):
                    extra[ei]()
                    ei += 1

            def emit_scores_qt(gt_s, xt_s, qt, e_pair, r_pair):
                """scores + exp + rowsum + reciprocal for one q-tile, writing
                the qt%2 half of the fp8 pair tiles.  ep-outer: one
                LDWEIGHTS per gt stationary, 4 kc matmuls reusing it, into
                two [P, 1024] PSUM tiles; exp runs once per kc pair."""
                j = qt % 2
                rsum = soft_pool.tile([P, 2], F32, tag="rsum")
                spa = sc_psum.tile([P, 2 * NCH], F32, tag="sc")
                spb = sc_psum.tile([P, 2 * NCH], F32, tag="sc")
                for ep in range(N_DP):
                    st_w = gt_s[:, 2 * ep : 2 * ep + 2, qt * P : (qt + 1) * P]
                    for kc in range(N_KC):
                        sp = spa if kc < 2 else spb
                        off = (kc % 2) * NCH
                        nc.tensor.matmul(
                            sp[:, off : off + NCH],
                            st_w,
                            xt_s[:, 2 * ep : 2 * ep + 2, kc * NCH : (kc + 1) * NCH],
                            start=(ep == 0),
                            stop=(ep == N_DP - 1),
                            perf_mode=DR,
                            skip_group_check=True,
                        )
                for h, sp in enumerate((spa, spb)):
                    nc.scalar.activation(
                        e_pair[:, j, 2 * h * NCH : 2 * (h + 1) * NCH],
                        sp[:],
                        EXP,
                        scale=SCALE / M_SCALE,
                        bias=ebias_t[:, 0:1],
                        accum_out=rsum[:, h : h + 1],
                    )
                rtot = soft_pool.tile([P, 1], F32, tag="rtot")
                nc.vector.reduce_sum(rtot[:], rsum[:], axis=mybir.AxisListType.X)
                rrec = soft_pool.tile([P, 1], F32, tag="rrec")
                nc.vector.reciprocal(rrec[:], rtot[:])
                # Broadcast 256*r across a [P, 128] fp8 stationary half so the
                # DoubleRow colsum runs at the full-width issue rate.
                nc.vector.tensor_scalar_mul(
                    r_pair[:, j, :], rrec[:, 0:1].broadcast_to([P, P]), R_SCALE
                )

            def emit_colsum_pair(w_ps, e_pair, r_pair, pair, kc0):
                """w_ps[:, kc-kc0, :] += bcast(256 r)^T @ E for a q-tile pair
                over kc in {kc0, kc0+1} (every output row is the colsum)."""
                for kk in range(2):
                    nc.tensor.matmul(
                        w_ps[:, kk, :],
                        r_pair[:, 0:2, :],
                        e_pair[:, 0:2, (kc0 + kk) * NCH : (kc0 + kk + 1) * NCH],
                        start=(pair == 0),
                        stop=(pair == N_QP - 1),
                        perf_mode=DR,
                        skip_group_check=True,
                    )

            def phase_scores(b, gt_s, xt_s, per_qt_extra=None, colsum_in_phase=True):
                """q-tile loop: scores+exp per qt.  kc 0-1 colsums either
                weave per completed pair, or (colsum_in_phase=False) are
                deferred entirely to the caller's next phase — freeing
                ~0.45us/q-tile of PE slack for woven extras.  Extras are
                spread evenly over the q-tiles."""
                w01_ps = None
                if colsum_in_phase:
                    w01_ps = w_psum.tile([P, 2, NCH], F32, tag="w")
                e_pairs, r_pairs = [], []
                pend = None
                extras = list(per_qt_extra) if per_qt_extra else []
                ei = 0
                e_pair = r_pair = None
                for qt in range(N_ST):
                    if qt % 2 == 0:
                        e_pair = e_pool.tile([P, 2, S], F8, tag="e")
                        r_pair = soft_pool.tile([P, 2, P], F8, tag="r", bufs=10)
                        e_pairs.append(e_pair)
                        r_pairs.append(r_pair)
                    emit_scores_qt(gt_s, xt_s, qt, e_pair, r_pair)
                    if colsum_in_phase and qt % 2 == 1:
                        # defer one pair so the colsum stationary swap lands
                        # while the next pair's scores stream
                        if pend is not None:
                            emit_colsum_pair(w01_ps, *pend, 0)
                        pend = (e_pair, r_pair, qt // 2)
                    want = ((qt + 1) * len(extras)) // N_ST
                    while ei < want:
                        extras[ei]()
                        ei += 1
                if pend is not None:
                    emit_colsum_pair(w01_ps, *pend, 0)
                return w01_ps, e_pairs, r_pairs

            def final_thunks(b, w01_ps, e_pairs, r_pairs, xnat_s):
                """(optionally the kc 0-1 colsums, when the scores phase
                deferred them) + kc 2-3 colsum sweep + w-phase: copies of
                256w, 16 (PE row->column transpose + broadcast), 16
                y-accumulation matmuls y = w @ X (bf16, in the free sc
                ring), then the epilogue y @ W_v and the output copy
                (x 1/256) + DMA."""
                w_sb = wvec_pool.tile([1, S], BF16, tag="wsb")
                wt_pads = {}
                yt_pads = {}
                thunks = []
                w01_box = {}
                if w01_ps is not None:
                    w01_box["t"] = w01_ps
                w23_box = {}
                y_box = {}

                def make_wcopy(w_box_key, kc, dst_kc):
                    def th():
                        src = w01_box["t"] if w_box_key is None else w23_box["t"]
                        eng = nc.scalar.copy if kc % 2 == 0 else nc.vector.tensor_copy
                        eng(
                            w_sb[:, dst_kc * NCH : (dst_kc + 1) * NCH],
                            src[0:1, kc, :],
                        )

                    return th

                def make_colsum01(pair):
                    def th():
                        if "t" not in w01_box:
                            w01_box["t"] = w_psum.tile(
                                [P, 2, NCH], F32, tag="w", name="w01d_ps"
                            )
                        emit_colsum_pair(
                            w01_box["t"], e_pairs[pair], r_pairs[pair], pair, 0
                        )

                    return th

                def make_sweep(pair):
                    def th():
                        if "t" not in w23_box:
                            w23_box["t"] = w_psum.tile(
                                [P, 2, NCH], F32, tag="w", name="w23_ps"
                            )
                        emit_colsum_pair(
                            w23_box["t"], e_pairs[pair], r_pairs[pair], pair, 2
                        )

                    return th

                def row_to_bcast_cols(src_row, pads, key, tag):
                    """[1,128] SBUF row chunk -> K=1 matmul -> [128,1] PSUM
                    column -> DVE broadcast to a [128,128] stationary tile."""
                    tp = gp_psum.tile([P, 1], F32, tag="gp")
                    nc.tensor.matmul(
                        tp[:], src_row, one_t[0:1, 0:1], start=True, stop=True
                    )
                    pad = wvec_pool.tile([P, P], BF16, tag=tag)
                    nc.vector.tensor_copy(pad[:], tp[:, 0:1].broadcast_to([P, P]))
                    pads[key] = pad

                def make_wtrans(kt):
                    def th():
                        row_to_bcast_cols(
                            w_sb[0:1, kt * P : (kt + 1) * P],
                            wt_pads, kt, f"wtp{kt % 4}",
                        )

                    return th

                def make_ymm(st):
                    def th():
                        # y accumulates in the sc ring (free outside the
                        # scores q-tile loop) so ymm needn't queue behind the
                        # w slot's sweep drain.
                        if "t" not in y_box:
                            y_box["t"] = sc_psum.tile(
                                [P, NCH], F32, tag="sc", name="y_ps"
                            )
                        nc.tensor.matmul(
                            y_box["t"][:],
                            wt_pads[st][:],
                            xnat_s[:, st, :],
                            start=(st == 0),
                            stop=(st == N_ST - 1),
                            skip_group_check=True,
                        )

                    return th

                def epilogue_th():
                    # y [1, D] -> o = y @ W_v  (4 K=1 transposes + 4 matmuls)
                    y_sb = wvec_pool.tile([1, NCH], BF16, tag="ysb")
                    nc.scalar.copy(y_sb[:], y_box["t"][0:1, :])
                    o_ps = gp_psum.tile([P, NCH], F32, tag="gp")
                    for c in range(N_DT):
                        row_to_bcast_cols(
                            y_sb[0:1, c * P : (c + 1) * P], yt_pads, c, f"ytp{c}"
                        )
                    for c in range(N_DT):
                        nc.tensor.matmul(
                            o_ps[:],
                            yt_pads[c][:],
                            wv_s[:, c, :],
                            start=(c == 0),
                            stop=(c == N_DT - 1),
                            skip_group_check=True,
                        )
                    o_sb = wvec_pool.tile([1, NCH], F32, tag="osb")
                    nc.scalar.mul(o_sb[:], o_ps[0:1, :], 1.0 / R_SCALE)
                    nc.sync.dma_start(out=out_ext[b : b + 1, :], in_=o_sb[:])

                # K=1 transposes batched apart from sweeps and ymm runs so
                # neither the sweep's r stationaries nor the ymm pads reload
                # mid-stream.
                if w01_ps is None:
                    for pair in range(N_QP):
                        thunks.append(make_colsum01(pair))
                for kc in range(2):
                    thunks.append(make_wcopy(None, kc, kc))
                for kt in range(N_QP):
                    thunks.append(make_wtrans(kt))
                for pair in range(N_QP):
                    thunks.append(make_sweep(pair))
                for kc in range(2):
                    thunks.append(make_wcopy("w23", kc, 2 + kc))
                for st in range(4):
                    thunks.append(make_ymm(st))
                for kt in range(N_QP, N_QP + 4):
                    thunks.append(make_wtrans(kt))
                for st in range(4, N_QP):
                    thunks.append(make_ymm(st))
                for kt in range(N_QP + 4, N_ST):
                    thunks.append(make_wtrans(kt))
                for st in range(N_QP, N_ST):
                    thunks.append(make_ymm(st))
                thunks.append(epilogue_th)
                return thunks

            # ------------------------- emission ------------------------------

            # batch 0: M prework + transposes woven into the G projection
            h0, dma0, trans0, kq0, mkT0, mkG0 = proj_thunks(0, xnat0_s, x0_loaded)
            g0, xt0 = h0
            if dma0[0] is not None:
                dma0[0]()
                dma0[0] = None

            def first_tile_trans_f32():
                tp = sc_psum.tile([P, N_DT * P], F32, tag="sc")
                for dt_i in range(N_DT):
                    nc.tensor.matmul(
                        tp[:, dt_i * P : (dt_i + 1) * P],
                        xf0[:, dt_i * P : (dt_i + 1) * P],
                        ident_f[:],
                        start=True,
                        stop=True,
                        skip_group_check=True,
                    )
                nc.scalar.copy(
                    xt0[:, :, 0:P],
                    tp[:].rearrange("p (t c) -> p t c", t=N_DT),
                )

            first_tile_trans_f32()
            for th in trans0[0][1:]:
                th()
            for th in m_prework_thunks():
                th()
            trans0 = [[], *trans0[1:]]
            emit_ltp(dma0, trans0, kq0)

            # batch 1's tiles + x DMAs go in before the scores phase (the
            # SWDGE queue streams them behind batch 0's data); its
            # transposes and first-half G weave into batch 0's ACT-bound
            # scores q-tile loop via the idle gp PSUM ring — batch 0's kc
            # 0-1 colsums are deferred to the mid to free the PE slack.
            xnat1_s = xnat_pool.tile([P, N_ST, D], BF16, tag="xnat")
            h1, dma1, trans1, kq1, mkT1, mkG1 = proj_thunks(
                1, xnat1_s, [False] * N_SC
            )
            g1, xt1 = h1
            for sc in range(N_SC):
                if dma1[sc] is not None:
                    dma1[sc]()
                    dma1[sc] = None

            extras0 = (
                [mkT1(sc, ti, ring="gp", eng="dve") for sc in (0, 1) for ti in range(4)]
                + [mkG1(0, ct) for ct in range(N_DT)]
                + [mkT1(sc, ti, ring="gp", eng="dve") for sc in (2, 3) for ti in range(4)]
            )
            wps0, eps0, rps0 = phase_scores(
                0, g0, xt0, per_qt_extra=extras0, colsum_in_phase=False
            )

            # mid: batch 1's second-half G (consumed by q-tiles 8-15 of the
            # next phase, so it pipelines into it) + batch 0's whole w-phase
            for ct in range(N_DT):
                mkG1(1, ct)()
            for th in final_thunks(0, None, eps0, rps0, xnat0_s):
                th()

            wps1, eps1, rps1 = phase_scores(1, g1, xt1)

            for th in final_thunks(1, wps1, eps1, rps1, xnat1_s):
                th()

    nc.compile()
    return nc


_NC_CACHE = None


def _get_nc():
    global _NC_CACHE
    if _NC_CACHE is None:
        _NC_CACHE = build_nc()
    return _NC_CACHE


def make_in_maps(inputs, W_q, W_k, W_v):
    inputs = np.ascontiguousarray(np.asarray(inputs, dtype=np.float32))
    W_q = np.ascontiguousarray(np.asarray(W_q, dtype=np.float32))
    W_k = np.ascontiguousarray(np.asarray(W_k, dtype=np.float32))
    W_v = np.ascontiguousarray(np.asarray(W_v, dtype=np.float32))
    return [
        {
            "inputs": inputs[i * B_PER_CORE : (i + 1) * B_PER_CORE],
            "W_q": W_q,
            "W_k": W_k,
            "W_v": W_v,
        }
        for i in range(N_CORES)
    ]


def kernel(**inputs) -> np.ndarray:
    nc = _get_nc()
    in_maps = make_in_maps(
        inputs["inputs"], inputs["W_q"], inputs["W_k"], inputs["W_v"]
    )
    res = run_bass_kernel_spmd(nc, in_maps, core_ids=list(range(N_CORES)))
    return np.concatenate(
        [res.results[i]["out"] for i in range(N_CORES)], axis=0
    ).astype(np.float32)


# revision 43
# speedup vs baseline: 1.0027x; 1.0027x over previous
"""Trainium2 Bass kernel for batched single-head attention with seq-sum pooling.

Reference computation (B=16, S=2048, D=512, fp32):
    q = x @ W_q ; k = x @ W_k ; v = x @ W_v          per batch  [S, D]
    scores = q @ k.T / sqrt(D)                        [S, S]
    attn = softmax(scores, axis=-1)
    out_b = sum_s (attn @ v)[s, :]                    [D]

Algebraic restructures (carried over from the bf16 version):
1. The final sum over query positions commutes through both trailing
   matmuls: out_b = ((r^T E) @ x) @ W_v with E = exp(scores/sqrt(D)) and
   r[q] = 1/rowsum_q(E) — removes the [S,S]x[S,D] attention-value matmul
   AND the V projection.
2. scores = x M x^T with M = W_q W_k^T computed once per core — replaces
   both per-batch Q/K projections with a single G = x M projection.

fp8 DoubleRow on the three dominant matmul groups (scores, G projection,
colsum) virtualizes the PE to K=256 (two fp8 weights per cell):
- x^T and 16*M are stored fp8; G's PSUM result (16*G) downcasts to fp8.
- scores PSUM holds 16*(q.k); the exp activation folds the 1/16 into its
  scale and applies bias -2 so E = exp(s/sqrt(D) - 2) stays within TRN
  fp8e4's +-240 range (softmax normalization cancels the shift exactly).
- E lands fp8 in PAIRED q-tile tiles [P, 2, S] so the colsum runs as
  DoubleRow too; the softmax reciprocal r is scaled by 256 (fp8 subnormal
  floor) and divided back out in the epilogue copy.
- The y = w @ X matvec and epilogue stay bf16 (x natural in fp8 would put
  ~3.6% noise directly on the output path).  rel err ~8e-3 (tol 2e-2).

PE/ACT pipeline shape (the v1 fp8 kernel measured 103 DR matmuls paying a
serialized 213ns LDWEIGHTS and an ACT busy near the PE's):
- Scores run ep-OUTER per q-tile: one 256-col LDWEIGHTS per gt stationary,
  then 4 kc matmuls reusing it, into two [P, 1024] PSUM tiles (kc pairs).
- exp runs once per kc-pair ([P, 1024] across two PSUM banks), halving the
  per-instruction ACT fixed cost + accumulator-read count.
- PSUM is exactly 8 banks: scores 2x[P,1024] + gp 2x[P,512] + a single
  [P,1024] w slot.  The colsum accumulator therefore only holds kc 0-1
  during the q-tile loop; kc 2-3 run as a second sweep over the persistent
  fp8 E tiles, woven into the next phase alongside the w-phase (whose y
  accumulator rides the same w slot after the sweep drains).
- G projection reuses each 16M stationary across an s-chunk pair.

Sharding: pure data parallelism over batch — 2 batch elements per core on 8
NeuronCores, weights replicated, no collectives.  Host concatenates per-core
[2, D] outputs.
"""

import sys

sys.path.insert(0, "/opt/trn_rl_repo")

import numpy as np

import concourse.bass as bass
import concourse.mybir as mybir
import concourse.tile as tile
from concourse import bacc
from concourse.bass_utils import run_bass_kernel_spmd
from concourse.masks import make_identity

B, S, D = 16, 2048, 512
P = 128
N_CORES = 8
B_PER_CORE = B // N_CORES  # 2
SCALE = 1.0 / float(np.sqrt(D))

F32 = mybir.dt.float32
BF16 = mybir.dt.bfloat16
F8 = mybir.dt.float8e4
DR = mybir.MatmulPerfMode.DoubleRow
EXP = mybir.ActivationFunctionType.Exp

M_SCALE = 16.0  # M stored as 16*M in fp8 (keeps values in normal range)
R_SCALE = 256.0  # r stored as 256*r in fp8 (dodges the subnormal floor)
EXP_BIAS = -2.0  # E = exp(s/sqrt(D) - 2): max stays well under fp8e4's 240

N_ST = S // P  # 16 s-tiles (partition tiles of the sequence dim)
N_DT = D // P  # 4 d-tiles (partition tiles of the feature dim)
N_DP = N_DT // 2  # 2 DoubleRow d-tile pairs
NCH = 512  # one fp32 PSUM bank of moving free dim
N_SC = S // NCH  # 4 s-chunks of the sequence dim
N_KC = S // NCH  # 4 k-chunks of the key dim
N_QP = N_ST // 2  # 8 q-tile pairs (DoubleRow colsum granularity)


def build_nc():
    nc = bacc.Bacc("TRN2", target_bir_lowering=False, debug=False, num_devices=N_CORES)
    x_ext = nc.dram_tensor(
        "inputs", [B_PER_CORE, S, D], F32, kind="ExternalInput"
    ).ap()
    wq_ext = nc.dram_tensor("W_q", [D, D], F32, kind="ExternalInput").ap()
    wk_ext = nc.dram_tensor("W_k", [D, D], F32, kind="ExternalInput").ap()
    wv_ext = nc.dram_tensor("W_v", [D, D], F32, kind="ExternalInput").ap()
    out_ext = nc.dram_tensor("out", [B_PER_CORE, D], F32, kind="ExternalOutput").ap()

    with tile.TileContext(nc) as tc:
        with (
            tc.tile_pool(name="const", bufs=1) as const_pool,
            tc.tile_pool(name="w", bufs=1) as w_pool,
            tc.tile_pool(name="xnat", bufs=2) as xnat_pool,
            tc.tile_pool(name="xt", bufs=2) as xt_pool,
            tc.tile_pool(name="qkv", bufs=2) as qkv_pool,
            tc.tile_pool(name="e", bufs=10) as e_pool,
            tc.tile_pool(name="soft", bufs=4) as soft_pool,
            tc.tile_pool(name="wvec", bufs=2) as wvec_pool,
            tc.tile_pool(name="scps", bufs=2, space="PSUM") as sc_psum,
            tc.tile_pool(name="gpps", bufs=2, space="PSUM") as gp_psum,
            tc.tile_pool(name="wps", bufs=1, space="PSUM") as w_psum,
        ):
            one_t = const_pool.tile([1, 1], BF16)
            nc.gpsimd.memset(one_t[:], 1.0)
            ident_f = const_pool.tile([P, P], F32)
            make_identity(nc, ident_f[:])
            ident = const_pool.tile([P, P], BF16)
            nc.vector.tensor_copy(ident[:], ident_f[:])
            ident8 = const_pool.tile([P, P], F8)
            nc.vector.tensor_copy(ident8[:], ident_f[:])
            ebias_t = const_pool.tile([P, 1], F32)
            nc.vector.memset(ebias_t[:], EXP_BIAS)

            # x arrives via SWDGE cast-DMA (f32 -> bf16) into natural-layout
            # staging tiles; the transpose to xT happens on the PE as a
            # REGULAR identity matmul (out = x_tile.T @ I) whose PSUM result
            # downcasts straight to fp8.
            def dma_x_chunk(b, sc, xnat_s):
                nc.gpsimd.dma_start(
                    out=xnat_s[:, sc * 4 : (sc + 1) * 4, :],
                    in_=x_ext[b, sc * NCH : (sc + 1) * NCH, :].rearrange(
                        "(t p) d -> p t d", p=P
                    ),
                )

            w_tiles = {}

            def dma_w(name, ext):
                w_s = w_pool.tile([P, N_DT, D], BF16, tag=name)
                nc.gpsimd.dma_start(
                    out=w_s[:], in_=ext.rearrange("(t p) e -> p t e", p=P)
                )
                w_tiles[name] = w_s

            # Batch 0's x chunks and the weight loads share the SWDGE queue;
            # order so each lands just before the PE needs it.
            xnat0_s = xnat_pool.tile([P, N_ST, D], BF16, tag="xnat")
            x0_loaded = [False] * N_SC
            # s-tile 0 rides the parallel HWDGE queue as f32 (lands ~3us
            # before the SWDGE cast chain's first byte); its transposes run
            # as fp32 identity matmuls.
            xf0 = xnat_pool.tile([P, D], F32, tag="xf0")
            nc.sync.dma_start(out=xf0[:], in_=x_ext[0, 0:P, :])
            nc.vector.tensor_copy(xnat0_s[:, 0, :], xf0[:])
            nc.gpsimd.dma_start(
                out=xnat0_s[:, 1:4, :],
                in_=x_ext[0, P:NCH, :].rearrange("(t p) d -> p t d", p=P),
            )
            x0_loaded[0] = True
            dma_w("wk", wk_ext)
            dma_w("wq", wq_ext)
            dma_x_chunk(0, 1, xnat0_s)
            x0_loaded[1] = True
            dma_x_chunk(0, 2, xnat0_s)
            x0_loaded[2] = True
            dma_x_chunk(0, 3, xnat0_s)
            x0_loaded[3] = True
            dma_w("wv", wv_ext)
            wk_s, wq_s, wv_s = w_tiles["wk"], w_tiles["wq"], w_tiles["wv"]

            # One-time prework: scores = (X Wq)(X Wk)^T = X M X^T with
            # M = Wq Wk^T [D, D].  M is computed in bf16 and stored as
            # 16*M in fp8 for the DoubleRow G projection.
            wqT_s = w_pool.tile([P, N_DT, D], BF16, tag="wqT")
            wkT_s = w_pool.tile([P, N_DT, D], BF16, tag="wkT")
            m_s = w_pool.tile([P, N_DT, D], F8, tag="m")

            def m_prework_thunks():
                thunks = []

                def make_wtrans_unit(src_w, dst, t_e):
                    def th():
                        tp = sc_psum.tile([P, N_DT * P], F32, tag="sc")
                        for t_a in range(N_DT):
                            nc.tensor.matmul(
                                tp[:, t_a * P : (t_a + 1) * P],
                                src_w[:, t_a, t_e * P : (t_e + 1) * P],
                                ident[:],
                                start=True,
                                stop=True,
                                skip_group_check=True,
                            )
                        nc.vector.tensor_copy(
                            dst[:, t_e, :],
                            tp[:],
                        )

                    return th

                def make_m_group(t_a):
                    def th():
                        mp = gp_psum.tile([P, NCH], F32, tag="gp")
                        for t_e in range(N_DT):
                            nc.tensor.matmul(
                                mp[:],
                                wqT_s[:, t_e, t_a * P : (t_a + 1) * P],
                                wkT_s[:, t_e, :],
                                start=(t_e == 0),
                                stop=(t_e == N_DT - 1),
                            )
                        nc.scalar.mul(m_s[:, t_a, :], mp[:], M_SCALE)

                    return th

                for t_e in range(N_DT):
                    thunks.append(make_wtrans_unit(wk_s, wkT_s, t_e))
                for t_e in range(N_DT):
                    thunks.append(make_wtrans_unit(wq_s, wqT_s, t_e))
                for t_a in range(N_DT):
                    thunks.append(make_m_group(t_a))
                return thunks

            # ---------- thunk builders (emission deferred for interleaving) --

            def proj_thunks(b, xnat_s, loaded):
                """Transpose + G = X M projection thunks for batch b.
                xt and gt land in fp8; the G PSUM holds 16*G (from 16*M).
                G groups cover an s-chunk PAIR so each 16M DoubleRow
                stationary is loaded once per two matmuls."""
                xt_s = xt_pool.tile([P, N_DT, S], F8, tag="xt")
                gt_s = qkv_pool.tile([P, N_DT, S], F8, tag="gt")

                def make_dma(sc):
                    def th():
                        dma_x_chunk(b, sc, xnat_s)

                    return th

                dma_th = [
                    None if loaded[sc] else make_dma(sc) for sc in range(N_SC)
                ]

                def make_trans_unit(sc, t_i, ring="sc", eng="act", src8=None):
                    def th():
                        st = sc * 4 + t_i
                        pool = sc_psum if ring == "sc" else gp_psum
                        tp = pool.tile([P, N_DT * P], F32, tag=ring, name="ttp")
                        # src8: an fp8 copy of x straight from a cast-DMA —
                        # numerically identical (xt is fp8 regardless) but
                        # the fp8 stationary's FWL weight load is 4x faster,
                        # which matters when woven into a dense PE stream.
                        src, idn = (
                            (xnat_s, ident) if src8 is None else (src8, ident8)
                        )
                        for dt_i in range(N_DT):
                            nc.tensor.matmul(
                                tp[:, dt_i * P : (dt_i + 1) * P],
                                src[:, st, dt_i * P : (dt_i + 1) * P],
                                idn[:],
                                start=True,
                                stop=True,
                                skip_group_check=True,
                            )
                        # ACT drains in the projection phases (DVE handles
                        # the G copies there); DVE when woven into the
                        # ACT-bound scores phases.
                        e = nc.scalar.copy if eng == "act" else nc.vector.tensor_copy
                        e(
                            xt_s[:, :, st * P : (st + 1) * P],
                            tp[:].rearrange("p (t c) -> p t c", t=N_DT),
                        )

                    return th

                trans_th = [
                    [make_trans_unit(sc, t_i) for t_i in range(4)]
                    for sc in range(N_SC)
                ]

                def make_g(scp, ct):
                    def th():
                        mpa = gp_psum.tile([P, NCH], F32, tag="gp")
                        mpb = gp_psum.tile([P, NCH], F32, tag="gp")
                        sca, scb = 2 * scp, 2 * scp + 1
                        for kp in range(N_DP):
                            st_w = m_s[:, 2 * kp : 2 * kp + 2, ct * P : (ct + 1) * P]
                            nc.tensor.matmul(
                                mpa[:],
                                st_w,
                                xt_s[:, 2 * kp : 2 * kp + 2, sca * NCH : (sca + 1) * NCH],
                                start=(kp == 0),
                                stop=(kp == N_DP - 1),
                                perf_mode=DR,
                            )
                            nc.tensor.matmul(
                                mpb[:],
                                st_w,
                                xt_s[:, 2 * kp : 2 * kp + 2, scb * NCH : (scb + 1) * NCH],
                                start=(kp == 0),
                                stop=(kp == N_DP - 1),
                                perf_mode=DR,
                            )
                        nc.vector.tensor_copy(
                            gt_s[:, ct, sca * NCH : (sca + 1) * NCH], mpa[:]
                        )
                        nc.vector.tensor_copy(
                            gt_s[:, ct, scb * NCH : (scb + 1) * NCH], mpb[:]
                        )

                    return th

                kq_th = [
                    [make_g(scp, ct) for ct in range(N_DT)]
                    for scp in range(N_SC // 2)
                ]
                return (gt_s, xt_s), dma_th, trans_th, kq_th, make_trans_unit, make_g

            def emit_ltp(dma_th, trans_th, kq_th, extra=None):
                """Emit the transpose/projection stream.  G groups span
                s-chunk pairs, so the order is: transposes for chunks
                2p,2p+1 -> G groups for pair p woven with the next pair's
                transposes.  `extra` thunks are spread evenly over all
                emission slots."""
                extra = list(extra) if extra else []
                slots = []  # flat list of thunks in emission order

                def emit_pair_dmas(p):
                    for j in (2 * p, 2 * p + 1):
                        if j < N_SC and dma_th[j] is not None:
                            dma_th[j]()
                            dma_th[j] = None

                emit_pair_dmas(0)
                for th in trans_th[0] + trans_th[1]:
                    slots.append(th)
                for scp in range(N_SC // 2):
                    if scp + 1 < N_SC // 2:
                        nxt = trans_th[2 * scp + 2] + trans_th[2 * scp + 3]
                    else:
                        nxt = []
                    groups = list(kq_th[scp])
                    ti = 0
                    for g_i, g in enumerate(groups):
                        if g_i == 0:
                            slots.append(lambda p=scp + 1: emit_pair_dmas(p))
                        slots.append(g)
                        while ti < len(nxt) and ti * len(groups) < (g_i + 1) * len(nxt):
                            slots.append(nxt[ti])
                            ti += 1
                    while ti < len(nxt):
                        slots.append(nxt[ti])
                        ti += 1

                n_slots = len(slots)
                ei = 0
                for s_i, th in enumerate(slots):
                    th()
                    # spread extras evenly across remaining slots
                    want = ((s_i + 1) * len(extra)) // n_slots
                    while ei < want:
                        extra[ei]()
                        ei += 1
                while ei < len(extra):
                    extra[ei]()
                    ei += 1

            def emit_scores_qt(gt_s, xt_s, qt, e_pair, r_pair):
                """scores + exp + rowsum + reciprocal for one q-tile, writing
                the qt%2 half of the fp8 pair tiles.  ep-outer: one
                LDWEIGHTS per gt stationary, 4 kc matmuls reusing it, into
                two [P, 1024] PSUM tiles; exp runs once per kc pair."""
                j = qt % 2
                rsum = soft_pool.tile([P, 2], F32, tag="rsum")
                spa = sc_psum.tile([P, 2 * NCH], F32, tag="sc")
                spb = sc_psum.tile([P, 2 * NCH], F32, tag="sc")
                for ep in range(N_DP):
                    st_w = gt_s[:, 2 * ep : 2 * ep + 2, qt * P : (qt + 1) * P]
                    for kc in range(N_KC):
                        sp = spa if kc < 2 else spb
                        off = (kc % 2) * NCH
                        nc.tensor.matmul(
                            sp[:, off : off + NCH],
                            st_w,
                            xt_s[:, 2 * ep : 2 * ep + 2, kc * NCH : (kc + 1) * NCH],
                            start=(ep == 0),
                            stop=(ep == N_DP - 1),
                            perf_mode=DR,
                            skip_group_check=True,
                        )
                for h, sp in enumerate((spa, spb)):
                    nc.scalar.activation(
                        e_pair[:, j, 2 * h * NCH : 2 * (h + 1) * NCH],
                        sp[:],
                        EXP,
                        scale=SCALE / M_SCALE,
                        bias=ebias_t[:, 0:1],
                        accum_out=rsum[:, h : h + 1],
                    )
                rtot = soft_pool.tile([P, 1], F32, tag="rtot")
                nc.vector.reduce_sum(rtot[:], rsum[:], axis=mybir.AxisListType.X)
                rrec = soft_pool.tile([P, 1], F32, tag="rrec")
                nc.vector.reciprocal(rrec[:], rtot[:])
                # Broadcast 256*r across a [P, 128] fp8 stationary half so the
                # DoubleRow colsum runs at the full-width issue rate.
                nc.vector.tensor_scalar_mul(
                    r_pair[:, j, :], rrec[:, 0:1].broadcast_to([P, P]), R_SCALE
                )

            def emit_colsum_pair(w_ps, e_pair, r_pair, pair, kc0):
                """w_ps[:, kc-kc0, :] += bcast(256 r)^T @ E for a q-tile pair
                over kc in {kc0, kc0+1} (every output row is the colsum)."""
                for kk in range(2):
                    nc.tensor.matmul(
                        w_ps[:, kk, :],
                        r_pair[:, 0:2, :],
                        e_pair[:, 0:2, (kc0 + kk) * NCH : (kc0 + kk + 1) * NCH],
                        start=(pair == 0),
                        stop=(pair == N_QP - 1),
                        perf_mode=DR,
                        skip_group_check=True,
                    )

            def phase_scores(b, gt_s, xt_s, per_qt_extra=None, colsum_in_phase=True):
                """q-tile loop: scores+exp per qt.  kc 0-1 colsums either
                weave per completed pair, or (colsum_in_phase=False) are
                deferred entirely to the caller's next phase — freeing
                ~0.45us/q-tile of PE slack for woven extras.  Extras are
                spread evenly over the q-tiles."""
                w01_ps = None
                if colsum_in_phase:
                    w01_ps = w_psum.tile([P, 2, NCH], F32, tag="w")
                e_pairs, r_pairs = [], []
                pend = None
                extras = list(per_qt_extra) if per_qt_extra else []
                ei = 0
                e_pair = r_pair = None
                for qt in range(N_ST):
                    if qt % 2 == 0:
                        e_pair = e_pool.tile([P, 2, S], F8, tag="e")
                        r_pair = soft_pool.tile([P, 2, P], F8, tag="r", bufs=10)
                        e_pairs.append(e_pair)
                        r_pairs.append(r_pair)
                    emit_scores_qt(gt_s, xt_s, qt, e_pair, r_pair)
                    if colsum_in_phase and qt % 2 == 1:
                        # defer one pair so the colsum stationary swap lands
                        # while the next pair's scores stream
                        if pend is not None:
                            emit_colsum_pair(w01_ps, *pend, 0)
                        pend = (e_pair, r_pair, qt // 2)
                    want = ((qt + 1) * len(extras)) // N_ST
                    while ei < want:
                        extras[ei]()
                        ei += 1
                if pend is not None:
                    emit_colsum_pair(w01_ps, *pend, 0)
                return w01_ps, e_pairs, r_pairs

            def final_thunks(b, w01_ps, e_pairs, r_pairs, xnat_s):
                """(optionally the kc 0-1 colsums, when the scores phase
                deferred them) + kc 2-3 colsum sweep + w-phase: copies of
                256w, 16 (PE row->column transpose + broadcast), 16
                y-accumulation matmuls y = w @ X (bf16, in the free sc
                ring), then the epilogue y @ W_v and the output copy
                (x 1/256) + DMA."""
                w_sb = wvec_pool.tile([1, S], BF16, tag="wsb")
                wt_pads = {}
                yt_pads = {}
                thunks = []
                w01_box = {}
                if w01_ps is not None:
                    w01_box["t"] = w01_ps
                w23_box = {}
                y_box = {}

                def make_wcopy(w_box_key, kc, dst_kc):
                    def th():
                        src = w01_box["t"] if w_box_key is None else w23_box["t"]
                        eng = nc.scalar.copy if kc % 2 == 0 else nc.vector.tensor_copy
                        eng(
                            w_sb[:, dst_kc * NCH : (dst_kc + 1) * NCH],
                            src[0:1, kc, :],
                        )

                    return th

                def make_colsum01(pair):
                    def th():
                        if "t" not in w01_box:
                            w01_box["t"] = w_psum.tile(
                                [P, 2, NCH], F32, tag="w", name="w01d_ps"
                            )
                        emit_colsum_pair(
                            w01_box["t"], e_pairs[pair], r_pairs[pair], pair, 0
                        )

                    return th

                def make_sweep(pair):
                    def th():
                        if "t" not in w23_box:
                            w23_box["t"] = w_psum.tile(
                                [P, 2, NCH], F32, tag="w", name="w23_ps"
                            )
                        emit_colsum_pair(
                            w23_box["t"], e_pairs[pair], r_pairs[pair], pair, 2
                        )

                    return th

                def row_to_bcast_cols(src_row, pads, key, tag):
                    """[1,128] SBUF row chunk -> K=1 matmul -> [128,1] PSUM
                    column -> DVE broadcast to a [128,128] stationary tile."""
                    tp = gp_psum.tile([P, 1], F32, tag="gp")
                    nc.tensor.matmul(
                        tp[:], src_row, one_t[0:1, 0:1], start=True, stop=True
                    )
                    pad = wvec_pool.tile([P, P], BF16, tag=tag)
                    nc.vector.tensor_copy(pad[:], tp[:, 0:1].broadcast_to([P, P]))
                    pads[key] = pad

                def make_wtrans(kt):
                    def th():
                        row_to_bcast_cols(
                            w_sb[0:1, kt * P : (kt + 1) * P],
                            wt_pads, kt, f"wtp{kt % 4}",
                        )

                    return th

                def make_ymm(st):
                    def th():
                        # y accumulates in the sc ring (free outside the
                        # scores q-tile loop) so ymm needn't queue behind the
                        # w slot's sweep drain.
                        if "t" not in y_box:
                            y_box["t"] = sc_psum.tile(
                                [P, NCH], F32, tag="sc", name="y_ps"
                            )
                        nc.tensor.matmul(
                            y_box["t"][:],
                            wt_pads[st][:],
                            xnat_s[:, st, :],
                            start=(st == 0),
                            stop=(st == N_ST - 1),
                            skip_group_check=True,
                        )

                    return th

                def epilogue_th():
                    # y [1, D] -> o = y @ W_v  (4 K=1 transposes + 4 matmuls)
                    y_sb = wvec_pool.tile([1, NCH], BF16, tag="ysb")
                    nc.scalar.copy(y_sb[:], y_box["t"][0:1, :])
                    o_ps = gp_psum.tile([P, NCH], F32, tag="gp")
                    for c in range(N_DT):
                        row_to_bcast_cols(
                            y_sb[0:1, c * P : (c + 1) * P], yt_pads, c, f"ytp{c}"
                        )
                    for c in range(N_DT):
                        nc.tensor.matmul(
                            o_ps[:],
                            yt_pads[c][:],
                            wv_s[:, c, :],
                            start=(c == 0),
                            stop=(c == N_DT - 1),
                            skip_group_check=True,
                        )
                    o_sb = wvec_pool.tile([1, NCH], F32, tag="osb")
                    nc.scalar.mul(o_sb[:], o_ps[0:1, :], 1.0 / R_SCALE)
                    nc.sync.dma_start(out=out_ext[b : b + 1, :], in_=o_sb[:])

                # K=1 transposes batched apart from sweeps and ymm runs so
                # neither the sweep's r stationaries nor the ymm pads reload
                # mid-stream.
                if w01_ps is None:
                    for pair in range(N_QP):
                        thunks.append(make_colsum01(pair))
                for kc in range(2):
                    thunks.append(make_wcopy(None, kc, kc))
                for kt in range(N_QP):
                    thunks.append(make_wtrans(kt))
                for pair in range(N_QP):
                    thunks.append(make_sweep(pair))
                for kc in range(2):
                    thunks.append(make_wcopy("w23", kc, 2 + kc))
                for st in range(4):
                    thunks.append(make_ymm(st))
                for kt in range(N_QP, N_QP + 4):
                    thunks.append(make_wtrans(kt))
                for st in range(4, N_QP):
                    thunks.append(make_ymm(st))
                for kt in range(N_QP + 4, N_ST):
                    thunks.append(make_wtrans(kt))
                for st in range(N_QP, N_ST):
                    thunks.append(make_ymm(st))
                thunks.append(epilogue_th)
                return thunks

            # ------------------------- emission ------------------------------

            # batch 0: M prework + transposes woven into the G projection
            h0, dma0, trans0, kq0, mkT0, mkG0 = proj_thunks(0, xnat0_s, x0_loaded)
            g0, xt0 = h0
            if dma0[0] is not None:
                dma0[0]()
                dma0[0] = None

            def first_tile_trans_f32():
                tp = sc_psum.tile([P, N_DT * P], F32, tag="sc")
                for dt_i in range(N_DT):
                    nc.tensor.matmul(
                        tp[:, dt_i * P : (dt_i + 1) * P],
                        xf0[:, dt_i * P : (dt_i + 1) * P],
                        ident_f[:],
                        start=True,
                        stop=True,
                        skip_group_check=True,
                    )
                nc.scalar.copy(
                    xt0[:, :, 0:P],
                    tp[:].rearrange("p (t c) -> p t c", t=N_DT),
                )

            first_tile_trans_f32()
            for th in trans0[0][1:]:
                th()
            for th in m_prework_thunks():
                th()
            trans0 = [[], *trans0[1:]]
            emit_ltp(dma0, trans0, kq0)

            # batch 1's tiles + x DMAs go in before the scores phase (the
            # SWDGE queue streams them behind batch 0's data); its
            # transposes and first-half G weave into batch 0's ACT-bound
            # scores q-tile loop via the idle gp PSUM ring — batch 0's kc
            # 0-1 colsums are deferred to the mid to free the PE slack.
            xnat1_s = xnat_pool.tile([P, N_ST, D], BF16, tag="xnat")
            h1, dma1, trans1, kq1, mkT1, mkG1 = proj_thunks(
                1, xnat1_s, [True] * N_SC
            )
            g1, xt1 = h1
            # batch 1's x streams TWICE on the otherwise-idle DMA engines:
            # first as fp8 chunks (feeding the woven transposes just in
            # time), then as one bf16 whole-batch copy (only needed by the
            # w-phase matvec in the mid).
            xnat1_8 = xnat_pool.tile([P, N_ST, D], F8, tag="x8", bufs=1)
            for sc in range(N_SC):
                nc.gpsimd.dma_start(
                    out=xnat1_8[:, sc * 4 : (sc + 1) * 4, :],
                    in_=x_ext[1, sc * NCH : (sc + 1) * NCH, :].rearrange(
                        "(t p) d -> p t d", p=P
                    ),
                )
            nc.gpsimd.dma_start(
                out=xnat1_s[:],
                in_=x_ext[1].rearrange("(t p) d -> p t d", p=P),
            )

            extras0 = (
                [
                    mkT1(sc, ti, ring="gp", eng="dve", src8=xnat1_8)
                    for sc in (0, 1)
                    for ti in range(4)
                ]
                + [mkG1(0, ct) for ct in range(N_DT)]
                + [
                    mkT1(sc, ti, ring="gp", eng="dve", src8=xnat1_8)
                    for sc in (2, 3)
                    for ti in range(4)
                ]
            )
            wps0, eps0, rps0 = phase_scores(
                0, g0, xt0, per_qt_extra=extras0, colsum_in_phase=False
            )

            # mid: batch 1's second-half G (consumed by q-tiles 8-15 of the
            # next phase, so it pipelines into it) + batch 0's whole w-phase
            for ct in range(N_DT):
                mkG1(1, ct)()
            for th in final_thunks(0, None, eps0, rps0, xnat0_s):
                th()

            wps1, eps1, rps1 = phase_scores(1, g1, xt1)

            for th in final_thunks(1, wps1, eps1, rps1, xnat1_s):
                th()

    nc.compile()
    return nc


_NC_CACHE = None


def _get_nc():
    global _NC_CACHE
    if _NC_CACHE is None:
        _NC_CACHE = build_nc()
    return _NC_CACHE


def make_in_maps(inputs, W_q, W_k, W_v):
    inputs = np.ascontiguousarray(np.asarray(inputs, dtype=np.float32))
    W_q = np.ascontiguousarray(np.asarray(W_q, dtype=np.float32))
    W_k = np.ascontiguousarray(np.asarray(W_k, dtype=np.float32))
    W_v = np.ascontiguousarray(np.asarray(W_v, dtype=np.float32))
    return [
        {
            "inputs": inputs[i * B_PER_CORE : (i + 1) * B_PER_CORE],
            "W_q": W_q,
            "W_k": W_k,
            "W_v": W_v,
        }
        for i in range(N_CORES)
    ]


def kernel(**inputs) -> np.ndarray:
    nc = _get_nc()
    in_maps = make_in_maps(
        inputs["inputs"], inputs["W_q"], inputs["W_k"], inputs["W_v"]
    )
    res = run_bass_kernel_spmd(nc, in_maps, core_ids=list(range(N_CORES)))
    return np.concatenate(
        [res.results[i]["out"] for i in range(N_CORES)], axis=0
    ).astype(np.float32)


# revision 44
# speedup vs baseline: 1.0341x; 1.0313x over previous
"""Trainium2 Bass kernel for batched single-head attention with seq-sum pooling.

Reference computation (B=16, S=2048, D=512, fp32):
    q = x @ W_q ; k = x @ W_k ; v = x @ W_v          per batch  [S, D]
    scores = q @ k.T / sqrt(D)                        [S, S]
    attn = softmax(scores, axis=-1)
    out_b = sum_s (attn @ v)[s, :]                    [D]

Algebraic restructures (carried over from the bf16 version):
1. The final sum over query positions commutes through both trailing
   matmuls: out_b = ((r^T E) @ x) @ W_v with E = exp(scores/sqrt(D)) and
   r[q] = 1/rowsum_q(E) — removes the [S,S]x[S,D] attention-value matmul
   AND the V projection.
2. scores = x M x^T with M = W_q W_k^T computed once per core — replaces
   both per-batch Q/K projections with a single G = x M projection.

fp8 DoubleRow on the three dominant matmul groups (scores, G projection,
colsum) virtualizes the PE to K=256 (two fp8 weights per cell):
- x^T and 16*M are stored fp8; G's PSUM result (16*G) downcasts to fp8.
- scores PSUM holds 16*(q.k); the exp activation folds the 1/16 into its
  scale and applies bias -2 so E = exp(s/sqrt(D) - 2) stays within TRN
  fp8e4's +-240 range (softmax normalization cancels the shift exactly).
- E lands fp8 in PAIRED q-tile tiles [P, 2, S] so the colsum runs as
  DoubleRow too; the softmax reciprocal r is scaled by 256 (fp8 subnormal
  floor) and divided back out in the epilogue copy.
- The y = w @ X matvec and epilogue stay bf16 (x natural in fp8 would put
  ~3.6% noise directly on the output path).  rel err ~8e-3 (tol 2e-2).

PE/ACT pipeline shape (the v1 fp8 kernel measured 103 DR matmuls paying a
serialized 213ns LDWEIGHTS and an ACT busy near the PE's):
- Scores run ep-OUTER per q-tile: one 256-col LDWEIGHTS per gt stationary,
  then 4 kc matmuls reusing it, into two [P, 1024] PSUM tiles (kc pairs).
- exp runs once per kc-pair ([P, 1024] across two PSUM banks), halving the
  per-instruction ACT fixed cost + accumulator-read count.
- PSUM is exactly 8 banks: scores 2x[P,1024] + gp 2x[P,512] + a single
  [P,1024] w slot.  The colsum accumulator therefore only holds kc 0-1
  during the q-tile loop; kc 2-3 run as a second sweep over the persistent
  fp8 E tiles, woven into the next phase alongside the w-phase (whose y
  accumulator rides the same w slot after the sweep drains).
- G projection reuses each 16M stationary across an s-chunk pair.

Sharding: pure data parallelism over batch — 2 batch elements per core on 8
NeuronCores, weights replicated, no collectives.  Host concatenates per-core
[2, D] outputs.
"""

import sys

sys.path.insert(0, "/opt/trn_rl_repo")

import numpy as np

import concourse.bass as bass
import concourse.mybir as mybir
import concourse.tile as tile
from concourse import bacc
from concourse.bass_utils import run_bass_kernel_spmd
from concourse.masks import make_identity

B, S, D = 16, 2048, 512
P = 128
N_CORES = 8
B_PER_CORE = B // N_CORES  # 2
SCALE = 1.0 / float(np.sqrt(D))

F32 = mybir.dt.float32
BF16 = mybir.dt.bfloat16
F8 = mybir.dt.float8e4
DR = mybir.MatmulPerfMode.DoubleRow
EXP = mybir.ActivationFunctionType.Exp

M_SCALE = 16.0  # M stored as 16*M in fp8 (keeps values in normal range)
R_SCALE = 256.0  # r stored as 256*r in fp8 (dodges the subnormal floor)
EXP_BIAS = -2.0  # E = exp(s/sqrt(D) - 2): max stays well under fp8e4's 240

N_ST = S // P  # 16 s-tiles (partition tiles of the sequence dim)
N_DT = D // P  # 4 d-tiles (partition tiles of the feature dim)
N_DP = N_DT // 2  # 2 DoubleRow d-tile pairs
NCH = 512  # one fp32 PSUM bank of moving free dim
N_SC = S // NCH  # 4 s-chunks of the sequence dim
N_KC = S // NCH  # 4 k-chunks of the key dim
N_QP = N_ST // 2  # 8 q-tile pairs (DoubleRow colsum granularity)


def build_nc():
    nc = bacc.Bacc("TRN2", target_bir_lowering=False, debug=False, num_devices=N_CORES)
    x_ext = nc.dram_tensor(
        "inputs", [B_PER_CORE, S, D], F32, kind="ExternalInput"
    ).ap()
    wq_ext = nc.dram_tensor("W_q", [D, D], F32, kind="ExternalInput").ap()
    wk_ext = nc.dram_tensor("W_k", [D, D], F32, kind="ExternalInput").ap()
    wv_ext = nc.dram_tensor("W_v", [D, D], F32, kind="ExternalInput").ap()
    out_ext = nc.dram_tensor("out", [B_PER_CORE, D], F32, kind="ExternalOutput").ap()

    with tile.TileContext(nc) as tc:
        with (
            tc.tile_pool(name="const", bufs=1) as const_pool,
            tc.tile_pool(name="w", bufs=1) as w_pool,
            tc.tile_pool(name="xnat", bufs=2) as xnat_pool,
            tc.tile_pool(name="xt", bufs=2) as xt_pool,
            tc.tile_pool(name="qkv", bufs=2) as qkv_pool,
            tc.tile_pool(name="e", bufs=10) as e_pool,
            tc.tile_pool(name="soft", bufs=4) as soft_pool,
            tc.tile_pool(name="wvec", bufs=2) as wvec_pool,
            tc.tile_pool(name="scps", bufs=2, space="PSUM") as sc_psum,
            tc.tile_pool(name="gpps", bufs=2, space="PSUM") as gp_psum,
            tc.tile_pool(name="wps", bufs=1, space="PSUM") as w_psum,
        ):
            one_t = const_pool.tile([1, 1], BF16)
            nc.gpsimd.memset(one_t[:], 1.0)
            ident_f = const_pool.tile([P, P], F32)
            make_identity(nc, ident_f[:])
            ident = const_pool.tile([P, P], BF16)
            nc.vector.tensor_copy(ident[:], ident_f[:])
            ident8 = const_pool.tile([P, P], F8)
            nc.vector.tensor_copy(ident8[:], ident_f[:])
            ebias_t = const_pool.tile([P, 1], F32)
            nc.vector.memset(ebias_t[:], EXP_BIAS)

            # x arrives via SWDGE cast-DMA (f32 -> bf16) into natural-layout
            # staging tiles; the transpose to xT happens on the PE as a
            # REGULAR identity matmul (out = x_tile.T @ I) whose PSUM result
            # downcasts straight to fp8.
            def dma_x_chunk(b, sc, xnat_s):
                nc.gpsimd.dma_start(
                    out=xnat_s[:, sc * 4 : (sc + 1) * 4, :],
                    in_=x_ext[b, sc * NCH : (sc + 1) * NCH, :].rearrange(
                        "(t p) d -> p t d", p=P
                    ),
                )

            w_tiles = {}

            def dma_w(name, ext):
                w_s = w_pool.tile([P, N_DT, D], BF16, tag=name)
                nc.gpsimd.dma_start(
                    out=w_s[:], in_=ext.rearrange("(t p) e -> p t e", p=P)
                )
                w_tiles[name] = w_s

            # Batch 0's x chunks and the weight loads share the SWDGE queue;
            # order so each lands just before the PE needs it.
            xnat0_s = xnat_pool.tile([P, N_ST, D], BF16, tag="xnat")
            x0_loaded = [False] * N_SC
            # s-tile 0 rides the parallel HWDGE queue as f32 (lands ~3us
            # before the SWDGE cast chain's first byte); its transposes run
            # as fp32 identity matmuls.
            xf0 = xnat_pool.tile([P, D], F32, tag="xf0")
            nc.sync.dma_start(out=xf0[:], in_=x_ext[0, 0:P, :])
            nc.vector.tensor_copy(xnat0_s[:, 0, :], xf0[:])
            nc.gpsimd.dma_start(
                out=xnat0_s[:, 1:4, :],
                in_=x_ext[0, P:NCH, :].rearrange("(t p) d -> p t d", p=P),
            )
            x0_loaded[0] = True
            dma_w("wk", wk_ext)
            dma_w("wq", wq_ext)
            dma_x_chunk(0, 1, xnat0_s)
            x0_loaded[1] = True
            dma_x_chunk(0, 2, xnat0_s)
            x0_loaded[2] = True
            dma_x_chunk(0, 3, xnat0_s)
            x0_loaded[3] = True
            dma_w("wv", wv_ext)
            wk_s, wq_s, wv_s = w_tiles["wk"], w_tiles["wq"], w_tiles["wv"]

            # One-time prework: scores = (X Wq)(X Wk)^T = X M X^T with
            # M = Wq Wk^T [D, D].  M is computed in bf16 and stored as
            # 16*M in fp8 for the DoubleRow G projection.
            wqT_s = w_pool.tile([P, N_DT, D], BF16, tag="wqT")
            wkT_s = w_pool.tile([P, N_DT, D], BF16, tag="wkT")
            m_s = w_pool.tile([P, N_DT, D], F8, tag="m")

            def m_prework_thunks():
                thunks = []

                def make_wtrans_unit(src_w, dst, t_e):
                    def th():
                        tp = sc_psum.tile([P, N_DT * P], F32, tag="sc")
                        for t_a in range(N_DT):
                            nc.tensor.matmul(
                                tp[:, t_a * P : (t_a + 1) * P],
                                src_w[:, t_a, t_e * P : (t_e + 1) * P],
                                ident[:],
                                start=True,
                                stop=True,
                                skip_group_check=True,
                            )
                        nc.vector.tensor_copy(
                            dst[:, t_e, :],
                            tp[:],
                        )

                    return th

                def make_m_group(t_a):
                    def th():
                        mp = gp_psum.tile([P, NCH], F32, tag="gp")
                        for t_e in range(N_DT):
                            nc.tensor.matmul(
                                mp[:],
                                wqT_s[:, t_e, t_a * P : (t_a + 1) * P],
                                wkT_s[:, t_e, :],
                                start=(t_e == 0),
                                stop=(t_e == N_DT - 1),
                            )
                        nc.scalar.mul(m_s[:, t_a, :], mp[:], M_SCALE)

                    return th

                for t_e in range(N_DT):
                    thunks.append(make_wtrans_unit(wk_s, wkT_s, t_e))
                for t_e in range(N_DT):
                    thunks.append(make_wtrans_unit(wq_s, wqT_s, t_e))
                for t_a in range(N_DT):
                    thunks.append(make_m_group(t_a))
                return thunks

            # ---------- thunk builders (emission deferred for interleaving) --

            def proj_thunks(b, xnat_s, loaded):
                """Transpose + G = X M projection thunks for batch b.
                xt and gt land in fp8; the G PSUM holds 16*G (from 16*M).
                G groups cover an s-chunk PAIR so each 16M DoubleRow
                stationary is loaded once per two matmuls."""
                xt_s = xt_pool.tile([P, N_DT, S], F8, tag="xt")
                gt_s = qkv_pool.tile([P, N_DT, S], F8, tag="gt")

                def make_dma(sc):
                    def th():
                        dma_x_chunk(b, sc, xnat_s)

                    return th

                dma_th = [
                    None if loaded[sc] else make_dma(sc) for sc in range(N_SC)
                ]

                def make_trans_unit(sc, t_i, ring="sc", eng="act", src8=None):
                    def th():
                        st = sc * 4 + t_i
                        pool = sc_psum if ring == "sc" else gp_psum
                        tp = pool.tile([P, N_DT * P], F32, tag=ring, name="ttp")
                        # src8: an fp8 copy of x straight from a cast-DMA —
                        # numerically identical (xt is fp8 regardless) but
                        # the fp8 stationary's FWL weight load is 4x faster,
                        # which matters when woven into a dense PE stream.
                        src, idn = (
                            (xnat_s, ident) if src8 is None else (src8, ident8)
                        )
                        for dt_i in range(N_DT):
                            nc.tensor.matmul(
                                tp[:, dt_i * P : (dt_i + 1) * P],
                                src[:, st, dt_i * P : (dt_i + 1) * P],
                                idn[:],
                                start=True,
                                stop=True,
                                skip_group_check=True,
                            )
                        # ACT drains in the projection phases (DVE handles
                        # the G copies there); DVE when woven into the
                        # ACT-bound scores phases.
                        e = nc.scalar.copy if eng == "act" else nc.vector.tensor_copy
                        e(
                            xt_s[:, :, st * P : (st + 1) * P],
                            tp[:].rearrange("p (t c) -> p t c", t=N_DT),
                        )

                    return th

                trans_th = [
                    [make_trans_unit(sc, t_i) for t_i in range(4)]
                    for sc in range(N_SC)
                ]

                def make_g(scp, ct):
                    def th():
                        mpa = gp_psum.tile([P, NCH], F32, tag="gp")
                        mpb = gp_psum.tile([P, NCH], F32, tag="gp")
                        sca, scb = 2 * scp, 2 * scp + 1
                        for kp in range(N_DP):
                            st_w = m_s[:, 2 * kp : 2 * kp + 2, ct * P : (ct + 1) * P]
                            nc.tensor.matmul(
                                mpa[:],
                                st_w,
                                xt_s[:, 2 * kp : 2 * kp + 2, sca * NCH : (sca + 1) * NCH],
                                start=(kp == 0),
                                stop=(kp == N_DP - 1),
                                perf_mode=DR,
                            )
                            nc.tensor.matmul(
                                mpb[:],
                                st_w,
                                xt_s[:, 2 * kp : 2 * kp + 2, scb * NCH : (scb + 1) * NCH],
                                start=(kp == 0),
                                stop=(kp == N_DP - 1),
                                perf_mode=DR,
                            )
                        nc.vector.tensor_copy(
                            gt_s[:, ct, sca * NCH : (sca + 1) * NCH], mpa[:]
                        )
                        nc.vector.tensor_copy(
                            gt_s[:, ct, scb * NCH : (scb + 1) * NCH], mpb[:]
                        )

                    return th

                kq_th = [
                    [make_g(scp, ct) for ct in range(N_DT)]
                    for scp in range(N_SC // 2)
                ]
                return (gt_s, xt_s), dma_th, trans_th, kq_th, make_trans_unit, make_g

            def emit_ltp(dma_th, trans_th, kq_th, extra=None):
                """Emit the transpose/projection stream.  G groups span
                s-chunk pairs, so the order is: transposes for chunks
                2p,2p+1 -> G groups for pair p woven with the next pair's
                transposes.  `extra` thunks are spread evenly over all
                emission slots."""
                extra = list(extra) if extra else []
                slots = []  # flat list of thunks in emission order

                def emit_pair_dmas(p):
                    for j in (2 * p, 2 * p + 1):
                        if j < N_SC and dma_th[j] is not None:
                            dma_th[j]()
                            dma_th[j] = None

                emit_pair_dmas(0)
                for th in trans_th[0] + trans_th[1]:
                    slots.append(th)
                for scp in range(N_SC // 2):
                    if scp + 1 < N_SC // 2:
                        nxt = trans_th[2 * scp + 2] + trans_th[2 * scp + 3]
                    else:
                        nxt = []
                    groups = list(kq_th[scp])
                    ti = 0
                    for g_i, g in enumerate(groups):
                        if g_i == 0:
                            slots.append(lambda p=scp + 1: emit_pair_dmas(p))
                        slots.append(g)
                        while ti < len(nxt) and ti * len(groups) < (g_i + 1) * len(nxt):
                            slots.append(nxt[ti])
                            ti += 1
                    while ti < len(nxt):
                        slots.append(nxt[ti])
                        ti += 1

                n_slots = len(slots)
                ei = 0
                for s_i, th in enumerate(slots):
                    th()
                    # spread extras evenly across remaining slots
                    want = ((s_i + 1) * len(extra)) // n_slots
                    while ei < want:
                        extra[ei]()
                        ei += 1
                while ei < len(extra):
                    extra[ei]()
                    ei += 1

            def emit_scores_qt(gt_s, xt_s, qt, e_pair, r_pair):
                """scores + exp + rowsum + reciprocal for one q-tile, writing
                the qt%2 half of the fp8 pair tiles.  ep-outer: one
                LDWEIGHTS per gt stationary, 4 kc matmuls reusing it, into
                two [P, 1024] PSUM tiles; exp runs once per kc pair."""
                j = qt % 2
                rsum = soft_pool.tile([P, 2], F32, tag="rsum")
                spa = sc_psum.tile([P, 2 * NCH], F32, tag="sc")
                spb = sc_psum.tile([P, 2 * NCH], F32, tag="sc")
                for ep in range(N_DP):
                    st_w = gt_s[:, 2 * ep : 2 * ep + 2, qt * P : (qt + 1) * P]
                    for kc in range(N_KC):
                        sp = spa if kc < 2 else spb
                        off = (kc % 2) * NCH
                        nc.tensor.matmul(
                            sp[:, off : off + NCH],
                            st_w,
                            xt_s[:, 2 * ep : 2 * ep + 2, kc * NCH : (kc + 1) * NCH],
                            start=(ep == 0),
                            stop=(ep == N_DP - 1),
                            perf_mode=DR,
                            skip_group_check=True,
                        )
                for h, sp in enumerate((spa, spb)):
                    nc.scalar.activation(
                        e_pair[:, j, 2 * h * NCH : 2 * (h + 1) * NCH],
                        sp[:],
                        EXP,
                        scale=SCALE / M_SCALE,
                        bias=ebias_t[:, 0:1],
                        accum_out=rsum[:, h : h + 1],
                    )
                rtot = soft_pool.tile([P, 1], F32, tag="rtot")
                nc.vector.reduce_sum(rtot[:], rsum[:], axis=mybir.AxisListType.X)
                rrec = soft_pool.tile([P, 1], F32, tag="rrec")
                nc.vector.reciprocal(rrec[:], rtot[:])
                # Broadcast 256*r across a [P, 128] fp8 stationary half so the
                # DoubleRow colsum runs at the full-width issue rate.
                nc.vector.tensor_scalar_mul(
                    r_pair[:, j, :], rrec[:, 0:1].broadcast_to([P, P]), R_SCALE
                )

            def emit_colsum_pair(w_ps, e_pair, r_pair, pair, kc0):
                """w_ps[:, kc-kc0, :] += bcast(256 r)^T @ E for a q-tile pair
                over kc in {kc0, kc0+1} (every output row is the colsum)."""
                for kk in range(2):
                    nc.tensor.matmul(
                        w_ps[:, kk, :],
                        r_pair[:, 0:2, :],
                        e_pair[:, 0:2, (kc0 + kk) * NCH : (kc0 + kk + 1) * NCH],
                        start=(pair == 0),
                        stop=(pair == N_QP - 1),
                        perf_mode=DR,
                        skip_group_check=True,
                    )

            def phase_scores(b, gt_s, xt_s, per_qt_extra=None, colsum_in_phase=True):
                """q-tile loop: scores+exp per qt.  kc 0-1 colsums either
                weave per completed pair, or (colsum_in_phase=False) are
                deferred entirely to the caller's next phase — freeing
                ~0.45us/q-tile of PE slack for woven extras.  Extras are
                spread evenly over the q-tiles."""
                w01_ps = None
                if colsum_in_phase:
                    w01_ps = w_psum.tile([P, 2, NCH], F32, tag="w")
                e_pairs, r_pairs = [], []
                pend = None
                extras = list(per_qt_extra) if per_qt_extra else []
                ei = 0
                e_pair = r_pair = None
                for qt in range(N_ST):
                    # extras go FIRST: their PSUM drains then sit ahead of
                    # this q-tile's softmax r-chain in the DVE FIFO, keeping
                    # the shared gp ring from backpressuring the PE
                    want = ((qt + 1) * len(extras)) // N_ST
                    while ei < want:
                        extras[ei]()
                        ei += 1
                    if qt % 2 == 0:
                        e_pair = e_pool.tile([P, 2, S], F8, tag="e")
                        r_pair = soft_pool.tile([P, 2, P], F8, tag="r", bufs=10)
                        e_pairs.append(e_pair)
                        r_pairs.append(r_pair)
                    emit_scores_qt(gt_s, xt_s, qt, e_pair, r_pair)
                    if colsum_in_phase and qt % 2 == 1:
                        # defer one pair so the colsum stationary swap lands
                        # while the next pair's scores stream
                        if pend is not None:
                            emit_colsum_pair(w01_ps, *pend, 0)
                        pend = (e_pair, r_pair, qt // 2)
                if pend is not None:
                    emit_colsum_pair(w01_ps, *pend, 0)
                return w01_ps, e_pairs, r_pairs

            def final_thunks(b, w01_ps, e_pairs, r_pairs, xnat_s):
                """(optionally the kc 0-1 colsums, when the scores phase
                deferred them) + kc 2-3 colsum sweep + w-phase: copies of
                256w, 16 (PE row->column transpose + broadcast), 16
                y-accumulation matmuls y = w @ X (bf16, in the free sc
                ring), then the epilogue y @ W_v and the output copy
                (x 1/256) + DMA."""
                w_sb = wvec_pool.tile([1, S], BF16, tag="wsb")
                wt_pads = {}
                yt_pads = {}
                thunks = []
                w01_box = {}
                if w01_ps is not None:
                    w01_box["t"] = w01_ps
                w23_box = {}
                y_box = {}

                def make_wcopy(w_box_key, kc, dst_kc):
                    def th():
                        src = w01_box["t"] if w_box_key is None else w23_box["t"]
                        eng = nc.scalar.copy if kc % 2 == 0 else nc.vector.tensor_copy
                        eng(
                            w_sb[:, dst_kc * NCH : (dst_kc + 1) * NCH],
                            src[0:1, kc, :],
                        )

                    return th

                def make_colsum01(pair):
                    def th():
                        if "t" not in w01_box:
                            w01_box["t"] = w_psum.tile(
                                [P, 2, NCH], F32, tag="w", name="w01d_ps"
                            )
                        emit_colsum_pair(
                            w01_box["t"], e_pairs[pair], r_pairs[pair], pair, 0
                        )

                    return th

                def make_sweep(pair):
                    def th():
                        if "t" not in w23_box:
                            w23_box["t"] = w_psum.tile(
                                [P, 2, NCH], F32, tag="w", name="w23_ps"
                            )
                        emit_colsum_pair(
                            w23_box["t"], e_pairs[pair], r_pairs[pair], pair, 2
                        )

                    return th

                def row_to_bcast_cols(src_row, pads, key, tag):
                    """[1,128] SBUF row chunk -> K=1 matmul -> [128,1] PSUM
                    column -> DVE broadcast to a [128,128] stationary tile."""
                    tp = gp_psum.tile([P, 1], F32, tag="gp")
                    nc.tensor.matmul(
                        tp[:], src_row, one_t[0:1, 0:1], start=True, stop=True
                    )
                    pad = wvec_pool.tile([P, P], BF16, tag=tag)
                    nc.vector.tensor_copy(pad[:], tp[:, 0:1].broadcast_to([P, P]))
                    pads[key] = pad

                def make_wtrans(kt):
                    def th():
                        row_to_bcast_cols(
                            w_sb[0:1, kt * P : (kt + 1) * P],
                            wt_pads, kt, f"wtp{kt % 4}",
                        )

                    return th

                def make_ymm(st):
                    def th():
                        # y accumulates in the sc ring (free outside the
                        # scores q-tile loop) so ymm needn't queue behind the
                        # w slot's sweep drain.
                        if "t" not in y_box:
                            y_box["t"] = sc_psum.tile(
                                [P, NCH], F32, tag="sc", name="y_ps"
                            )
                        nc.tensor.matmul(
                            y_box["t"][:],
                            wt_pads[st][:],
                            xnat_s[:, st, :],
                            start=(st == 0),
                            stop=(st == N_ST - 1),
                            skip_group_check=True,
                        )

                    return th

                def epilogue_th():
                    # y [1, D] -> o = y @ W_v  (4 K=1 transposes + 4 matmuls)
                    y_sb = wvec_pool.tile([1, NCH], BF16, tag="ysb")
                    nc.scalar.copy(y_sb[:], y_box["t"][0:1, :])
                    o_ps = gp_psum.tile([P, NCH], F32, tag="gp")
                    for c in range(N_DT):
                        row_to_bcast_cols(
                            y_sb[0:1, c * P : (c + 1) * P], yt_pads, c, f"ytp{c}"
                        )
                    for c in range(N_DT):
                        nc.tensor.matmul(
                            o_ps[:],
                            yt_pads[c][:],
                            wv_s[:, c, :],
                            start=(c == 0),
                            stop=(c == N_DT - 1),
                            skip_group_check=True,
                        )
                    o_sb = wvec_pool.tile([1, NCH], F32, tag="osb")
                    nc.scalar.mul(o_sb[:], o_ps[0:1, :], 1.0 / R_SCALE)
                    nc.sync.dma_start(out=out_ext[b : b + 1, :], in_=o_sb[:])

                # K=1 transposes batched apart from sweeps and ymm runs so
                # neither the sweep's r stationaries nor the ymm pads reload
                # mid-stream.
                if w01_ps is None:
                    for pair in range(N_QP):
                        thunks.append(make_colsum01(pair))
                for kc in range(2):
                    thunks.append(make_wcopy(None, kc, kc))
                for kt in range(N_QP):
                    thunks.append(make_wtrans(kt))
                for pair in range(N_QP):
                    thunks.append(make_sweep(pair))
                for kc in range(2):
                    thunks.append(make_wcopy("w23", kc, 2 + kc))
                for st in range(4):
                    thunks.append(make_ymm(st))
                for kt in range(N_QP, N_QP + 4):
                    thunks.append(make_wtrans(kt))
                for st in range(4, N_QP):
                    thunks.append(make_ymm(st))
                for kt in range(N_QP + 4, N_ST):
                    thunks.append(make_wtrans(kt))
                for st in range(N_QP, N_ST):
                    thunks.append(make_ymm(st))
                thunks.append(epilogue_th)
                return thunks

            # ------------------------- emission ------------------------------

            # batch 0: M prework + transposes woven into the G projection
            h0, dma0, trans0, kq0, mkT0, mkG0 = proj_thunks(0, xnat0_s, x0_loaded)
            g0, xt0 = h0
            if dma0[0] is not None:
                dma0[0]()
                dma0[0] = None

            def first_tile_trans_f32():
                tp = sc_psum.tile([P, N_DT * P], F32, tag="sc")
                for dt_i in range(N_DT):
                    nc.tensor.matmul(
                        tp[:, dt_i * P : (dt_i + 1) * P],
                        xf0[:, dt_i * P : (dt_i + 1) * P],
                        ident_f[:],
                        start=True,
                        stop=True,
                        skip_group_check=True,
                    )
                nc.scalar.copy(
                    xt0[:, :, 0:P],
                    tp[:].rearrange("p (t c) -> p t c", t=N_DT),
                )

            first_tile_trans_f32()
            for th in trans0[0][1:]:
                th()
            for th in m_prework_thunks():
                th()
            trans0 = [[], *trans0[1:]]
            emit_ltp(dma0, trans0, kq0)

            # batch 1's tiles + x DMAs go in before the scores phase (the
            # SWDGE queue streams them behind batch 0's data); its
            # transposes and first-half G weave into batch 0's ACT-bound
            # scores q-tile loop via the idle gp PSUM ring — batch 0's kc
            # 0-1 colsums are deferred to the mid to free the PE slack.
            xnat1_s = xnat_pool.tile([P, N_ST, D], BF16, tag="xnat")
            h1, dma1, trans1, kq1, mkT1, mkG1 = proj_thunks(
                1, xnat1_s, [True] * N_SC
            )
            g1, xt1 = h1
            # batch 1's x streams TWICE on the otherwise-idle DMA engines:
            # first as fp8 chunks (feeding the woven transposes just in
            # time), then as one bf16 whole-batch copy (only needed by the
            # w-phase matvec in the mid).
            xnat1_8 = xnat_pool.tile([P, N_ST, D], F8, tag="x8", bufs=1)
            for sc in range(N_SC):
                nc.gpsimd.dma_start(
                    out=xnat1_8[:, sc * 4 : (sc + 1) * 4, :],
                    in_=x_ext[1, sc * NCH : (sc + 1) * NCH, :].rearrange(
                        "(t p) d -> p t d", p=P
                    ),
                )
            nc.gpsimd.dma_start(
                out=xnat1_s[:],
                in_=x_ext[1].rearrange("(t p) d -> p t d", p=P),
            )

            extras0 = (
                [
                    mkT1(sc, ti, ring="gp", eng="dve", src8=xnat1_8)
                    for sc in (0, 1)
                    for ti in range(4)
                ]
                + [mkG1(0, ct) for ct in range(N_DT)]
                + [
                    mkT1(sc, ti, ring="gp", eng="dve", src8=xnat1_8)
                    for sc in (2, 3)
                    for ti in range(4)
                ]
            )
            wps0, eps0, rps0 = phase_scores(
                0, g0, xt0, per_qt_extra=extras0, colsum_in_phase=False
            )

            # mid: batch 1's second-half G (consumed by q-tiles 8-15 of the
            # next phase, so it pipelines into it) + batch 0's whole w-phase
            for ct in range(N_DT):
                mkG1(1, ct)()
            for th in final_thunks(0, None, eps0, rps0, xnat0_s):
                th()

            wps1, eps1, rps1 = phase_scores(1, g1, xt1)

            for th in final_thunks(1, wps1, eps1, rps1, xnat1_s):
                th()

    nc.compile()
    return nc


_NC_CACHE = None


def _get_nc():
    global _NC_CACHE
    if _NC_CACHE is None:
        _NC_CACHE = build_nc()
    return _NC_CACHE


def make_in_maps(inputs, W_q, W_k, W_v):
    inputs = np.ascontiguousarray(np.asarray(inputs, dtype=np.float32))
    W_q = np.ascontiguousarray(np.asarray(W_q, dtype=np.float32))
    W_k = np.ascontiguousarray(np.asarray(W_k, dtype=np.float32))
    W_v = np.ascontiguousarray(np.asarray(W_v, dtype=np.float32))
    return [
        {
            "inputs": inputs[i * B_PER_CORE : (i + 1) * B_PER_CORE],
            "W_q": W_q,
            "W_k": W_k,
            "W_v": W_v,
        }
        for i in range(N_CORES)
    ]


def kernel(**inputs) -> np.ndarray:
    nc = _get_nc()
    in_maps = make_in_maps(
        inputs["inputs"], inputs["W_q"], inputs["W_k"], inputs["W_v"]
    )
    res = run_bass_kernel_spmd(nc, in_maps, core_ids=list(range(N_CORES)))
    return np.concatenate(
        [res.results[i]["out"] for i in range(N_CORES)], axis=0
    ).astype(np.float32)
